# revision 67
# baseline (speedup 1.0000x reference)
"""CAB multi-head attention on 8 Trainium2 NeuronCores.

Sharding: fully data-parallel, core c -> (batch b = c//2, query-half = c%2).
Each core computes 256 query rows against all 512 keys of its batch.
No collectives. Host does transposes/packing; device does all FLOPs.

Program order is CAB-MLP first (needs only the small se/W1/W2/W3 loads)
so the PE streams while the bulk Q/K/V/weight DMAs land in the
background (issued off the otherwise-idle gpsimd queue).

Per-core layout conventions (features on partitions, tokens on free):
  QT/KT [E, t] bf16; V [s, e] bf16; scoresT/attnT [s, t] (softmax along
  partitions via one-hot-column matmuls, no max subtraction needed).
  CAB pairs i-major: h/h2 [(d, i%2), j] bf16, with [128,1024] two-bank
  PSUM tiles so elementwise ops cover two 512-wide halves at once.
  The CAB output is exponentiated on-chip (ebias = exp(temps*comp+b3))
  and PE-transposed into biasT [j, (jc, head, tt, i, i%2)], so phase 3
  applies the bias MULTIPLICATIVELY: attn = exp(scores) * ebias with a
  contiguous per-(head, jc) DVE multiply - no bias preload matmuls.
  Softmax denominators accumulate over all heads in one [16, t] PSUM
  via one-hot hsel matmuls; 1/sums is partition-broadcast with a
  selector matmul (rsel) and applied on the DVE; out-proj runs
  hp-outer so accumulation starts as soon as the first scaled AV
  chunk is ready.
"""
import sys

sys.path.insert(0, "/opt/trn_rl_repo")

import numpy as np
import ml_dtypes
from contextlib import ExitStack

import concourse.bacc as bacc
import concourse.tile as tile
from concourse import mybir
from concourse.bass_utils import run_bass_kernel_spmd

F32 = mybir.dt.float32
F32R = mybir.dt.float32r
BF16 = mybir.dt.bfloat16
AF = mybir.ActivationFunctionType
ALU = mybir.AluOpType

B, N, E, H, SD, HID = 4, 512, 1024, 16, 64, 64
D = E // H
NQ = 256            # query rows per core
NCORES = 8
NTT = NQ // 8       # 32 tt groups (4 i-pairs each) in the CAB stage

_BF = ml_dtypes.bfloat16


def _build_program(debug=False):
    nc = bacc.Bacc("TRN2", target_bir_lowering=False, debug=False,
                   num_devices=NCORES)

    def din(name, shape, dt):
        return nc.dram_tensor(name, list(shape), dt, kind="ExternalInput").ap()

    d = {}
    d["qT"] = din("qT", (E, NQ), BF16)
    d["kT"] = din("kT", (E, N), BF16)
    d["vT"] = din("vT", (E, N), BF16)
    d["seT"] = din("seT", (SD, N), F32R)
    d["seQ"] = din("seQ", (SD, NQ), F32R)
    d["wq"] = din("wq", (E, E), BF16)
    d["wk"] = din("wk", (E, E), BF16)
    d["wv"] = din("wv", (E, E), BF16)
    d["wo"] = din("wo", (E, E), BF16)
    d["w1a"] = din("w1a", (SD, 128), F32R)
    d["w1b"] = din("w1b", (SD, 128), F32R)
    d["w2bd"] = din("w2bd", (128, 128), BF16)
    d["w3bd"] = din("w3bd", (128, 32), BF16)
    d["id128"] = din("id128", (128, 128), BF16)
    d["hsel"] = din("hsel", (128, H * 16), BF16)
    d["bq128"] = din("bq128", (128, 8), F32)
    d["bk128"] = din("bk128", (128, 8), F32)
    d["b1d"] = din("b1d", (128, 1), F32)
    d["b2d"] = din("b2d", (128, 1), F32)
    d["t128"] = din("t128", (128, 1), F32)
    d["b3t"] = din("b3t", (128, 1), F32)
    d["bv2d"] = din("bv2d", (1, E), BF16)
    d["rsel"] = din("rsel", (8, 8 * 128), F32R)
    out_d = nc.dram_tensor("out", [NQ, E], F32, kind="ExternalOutput").ap()
    dbg = {}
    if debug:
        dbg["dQT"] = nc.dram_tensor("dQT", [128, NQ], F32, kind="ExternalOutput").ap()
        dbg["dKT"] = nc.dram_tensor("dKT", [128, N], F32, kind="ExternalOutput").ap()
        dbg["dV"] = nc.dram_tensor("dV", [128, 512], BF16, kind="ExternalOutput").ap()
        dbg["dhjT"] = nc.dram_tensor("dhjT", [128, N], BF16, kind="ExternalOutput").ap()
        dbg["dhiT"] = nc.dram_tensor("dhiT", [128, 128], F32, kind="ExternalOutput").ap()
        dbg["dbiasT"] = nc.dram_tensor("dbiasT", [128, NTT * 512], BF16, kind="ExternalOutput").ap()
        dbg["dat"] = nc.dram_tensor("dat", [128, NQ], BF16, kind="ExternalOutput").ap()
        dbg["dsums"] = nc.dram_tensor("dsums", [16, NQ], F32, kind="ExternalOutput").ap()
        dbg["davU"] = nc.dram_tensor("davU", [128, NQ], F32, kind="ExternalOutput").ap()
        dbg["davN"] = nc.dram_tensor("davN", [128, NQ], BF16, kind="ExternalOutput").ap()

    with tile.TileContext(nc) as tc, ExitStack() as ctx:
        # ---------------- persistent SBUF pools ----------------
        cst = ctx.enter_context(tc.tile_pool(name="cst", bufs=1))
        big = ctx.enter_context(tc.tile_pool(name="big", bufs=1))

        def cload(name, shape, dt, eng=None):
            t = cst.tile(list(shape), dt, tag=name, name=name)
            (eng or nc.sync).dma_start(t[:], d[name][:])
            return t

        # W1/CAB dependencies first: the sync engine issues these DMAs
        # serially (~0.6us each), and the CAB phase starts as soon as
        # they land.
        seT = cload("seT", (SD, N), F32R)
        seQ = cload("seQ", (SD, NQ), F32R)
        w1a = cload("w1a", (SD, 128), F32R)
        w1b = cload("w1b", (SD, 128), F32R)
        b1d = cload("b1d", (128, 1), F32)
        w2bd = cload("w2bd", (128, 128), BF16)
        b2d = cload("b2d", (128, 1), F32)
        w3bd = cload("w3bd", (128, 32), BF16)
        t128 = cload("t128", (128, 1), F32)
        b3t = cload("b3t", (128, 1), F32)
        id128 = cload("id128", (128, 128), BF16)
        # non-CAB consts ride the gpsimd issue queue (needed later)
        hsel = cload("hsel", (128, H * 16), BF16, eng=nc.gpsimd)
        bq128 = cload("bq128", (128, 8), F32, eng=nc.gpsimd)
        bk128 = cload("bk128", (128, 8), F32, eng=nc.gpsimd)
        rsel = cload("rsel", (8, 8 * 128), F32R, eng=nc.gpsimd)
        # V bias broadcast across all 128 s-partitions (stride-0 DMA)
        bvsb = cst.tile([128, E], BF16, tag="bvsb", name="bvsb")
        nc.gpsimd.dma_start(bvsb[:], d["bv2d"][0:1, :].broadcast_to([128, E]))

        # resident per-core inputs, chunked on k (one DMA each, k-chunk kc
        # of a [E, t] tensor lives in tile kc as [128, t]).  dma_start
        # issue is deferred until after the CAB loop is emitted so the
        # sync engine isn't clogged at t=0 (each issue costs ~600ns).
        def kchunks(name, t, dt, ntile=8):
            ts = []
            for k in range(ntile):
                tt = big.tile([128, t], dt, tag=f"{name}{k}", name=f"{name}{k}")
                ts.append(tt)
            return ts

        def kchunks_dma(name, ts):
            # issued from the (otherwise idle) gpsimd queue so the sync
            # engine isn't the serial bottleneck for bulk loads
            for k, tt in enumerate(ts):
                nc.gpsimd.dma_start(tt[:], d[name][k * 128:(k + 1) * 128, :])

        qTt = kchunks("qT", NQ, BF16)
        kTt = kchunks("kT", N, BF16)
        # Wv rows resident (rhs of V-proj), Wo rows resident (rhs of out-proj)
        wv_r = kchunks("wv", E, BF16)
        wo_r = kchunks("wo", E, BF16)

        # persistent intermediates
        QT = [big.tile([128, NQ], BF16, tag=f"QT{k}", name=f"QT{k}") for k in range(8)]
        KT = [big.tile([128, N], BF16, tag=f"KT{k}", name=f"KT{k}") for k in range(8)]
        Vsb = [[big.tile([128, 512], BF16, tag=f"V{st}_{et}", name=f"V{st}_{et}")
                for et in range(2)] for st in range(4)]
        hjT = big.tile([128, N], BF16, tag="hjT")
        hiT = big.tile([128, 128], F32, tag="hiT")
        biasT = big.tile([128, NTT * 512], BF16, tag="biasT")
        avU = [big.tile([128, NQ], F32, tag=f"avU{hp}", name=f"avU{hp}") for hp in range(8)]
        avN = [big.tile([128, NQ], BF16, tag=f"avN{hp}", name=f"avN{hp}") for hp in range(8)]
        sums_sb = big.tile([16, NQ], F32, tag="sums_sb")
        recip_sb = big.tile([16, NQ], F32R, tag="recip_sb")

        # ---------------- phase 0: W1 (only needs se + w1) ----------------
        with tc.tile_pool(name="w1ps", bufs=1, space="PSUM") as w1ps:
            # W1: hjT (dup'd, +b1, bf16) and hiT (packed by i-parity, f32)
            hj_ps = w1ps.tile([128, N], F32, tag="hjps")
            nc.tensor.matmul(hj_ps[:], w1b[:], seT[:], start=True, stop=True)
            nc.vector.tensor_scalar(hjT[:], hj_ps[:], b1d[:, 0:1], None,
                                    ALU.add)
            hi_ps = w1ps.tile([128, NQ], F32, tag="hips")
            nc.tensor.matmul(hi_ps[:], w1a[:], seQ[:], start=True, stop=True)
            hi_v = hi_ps[:].rearrange("p (i two) -> p i two", two=2)
            nc.vector.tensor_copy(hiT[0:64, :], hi_v[0:64, :, 0])
            nc.vector.tensor_copy(hiT[64:128, :], hi_v[64:128, :, 1])

        # ---------------- phase 2: CAB pair MLP + transpose ----------------
        # Runs first so the PE streams while the big Q/K/V/W DMAs land.
        # Works at tt-pair granularity: [128,1024] f32 PSUM tiles span two
        # banks, but each matmul writes a single-bank 512-col half, and the
        # wide elementwise ops amortize per-instruction fixed costs.
        with tc.tile_pool(name="hpool", bufs=4) as hpool, \
             tc.tile_pool(name="h2sb", bufs=4) as h2sbp, \
             tc.tile_pool(name="csb", bufs=2) as csbp, \
             tc.tile_pool(name="h2ps", bufs=2, space="PSUM") as h2ps, \
             tc.tile_pool(name="cps", bufs=1, space="PSUM") as cps, \
             tc.tile_pool(name="trps", bufs=2, space="PSUM") as trps:

            for ttp in range(NTT // 2):
                h2list = []
                for tt_h in range(2):
                    tt = 2 * ttp + tt_h
                    for half in range(2):
                        h_ts = []
                        for k in range(2):
                            ii = tt * 4 + half * 2 + k
                            h_t = hpool.tile([128, N], BF16, tag="h")
                            nc.vector.tensor_scalar(h_t[:], hjT[:],
                                                    hiT[:, ii:ii + 1], 0.0,
                                                    ALU.add, ALU.max)
                            h_ts.append(h_t)
                        ps = h2ps.tile([128, 2 * N], F32, tag="h2")
                        for k in range(2):
                            nc.tensor.matmul(ps[:, k * N:(k + 1) * N],
                                             w2bd[:], h_ts[k][:],
                                             start=True, stop=True)
                        h2big = h2sbp.tile([128, 2 * N], BF16, tag="h2sb")
                        pp = tt * 2 + half
                        if pp % 5 == 0:
                            nc.vector.tensor_scalar(h2big[:], ps[:],
                                                    b2d[:, 0:1], 0.0,
                                                    ALU.add, ALU.max)
                        else:
                            nc.scalar.activation(h2big[:], ps[:], AF.Relu,
                                                 bias=b2d[:, 0:1])
                        h2list.append(h2big)

                c_ps = cps.tile([128, 2 * N], F32, tag="comp")
                for tt_h in range(2):
                    for iic in range(4):
                        h2big = h2list[tt_h * 2 + iic // 2]
                        nc.tensor.matmul(
                            c_ps[32 * iic:32 * iic + 32,
                                 tt_h * N:(tt_h + 1) * N],
                            w3bd[:],
                            h2big[:, (iic % 2) * N:(iic % 2 + 1) * N],
                            start=True, stop=True,
                            tile_position=(0, 32 * iic))
                # ebias = exp(temps * comp + temps * b3): applied
                # multiplicatively to exp(scores) in phase 3, which saves
                # the per-(h,jc) bias-preload matmul into the scores PSUM.
                c_sb = csbp.tile([128, 2 * N], BF16, tag="csb")
                nc.scalar.activation(c_sb[:], c_ps[:], AF.Exp,
                                     bias=b3t[:, 0:1], scale=t128[:, 0:1])
                for tt_h in range(2):
                    tt = 2 * ttp + tt_h
                    tr_ps = trps.tile([128, 512], BF16, tag="tr")
                    for jc in range(4):
                        nc.tensor.transpose(
                            tr_ps[:, jc * 128:(jc + 1) * 128],
                            c_sb[:, tt_h * N + jc * 128:
                                 tt_h * N + (jc + 1) * 128],
                            id128[:])
                    # scatter tr_ps [j, (jc,i,x,m)] into biasT's
                    # (jc, x, t, i, m) layout: phase-3 reads contiguous
                    src = tr_ps[:].rearrange("p (j i x m) -> p j x i m",
                                             j=4, i=4, x=16, m=2)
                    dst = biasT[:].rearrange(
                        "p (j x t i m) -> p t j x i m",
                        j=4, x=16, t=NTT, i=4, m=2)[:, tt]
                    for jc in range(4):
                        nc.vector.tensor_copy(dst[:, jc], src[:, jc])
            if debug:
                nc.sync.dma_start(dbg["dbiasT"][:], biasT[:])

        # Q/K/V input + weight row chunks stream in while CAB runs.
        kchunks_dma("qT", qTt)
        kchunks_dma("kT", kTt)
        kchunks_dma("wv", wv_r)
        kchunks_dma("wo", wo_r)

        # ---------------- phase 1: Q/K/V projections ----------------
        with tc.tile_pool(name="wcol", bufs=3) as wcol, \
             tc.tile_pool(name="p1ps", bufs=3, space="PSUM") as p1ps:

            # Q/K projections: out-chunk ec outer, contraction kc inner.
            for ec in range(8):
                wq_c = wcol.tile([128, 1024], BF16, tag="wcol")
                nc.sync.dma_start(
                    wq_c[:],
                    d["wq"][:, ec * 128:(ec + 1) * 128]
                    .rearrange("(k p) c -> p k c", p=128))
                ps = p1ps.tile([128, 512], F32, tag="p1", name="qps")[:, 0:NQ]
                for kc in range(8):
                    nc.tensor.matmul(ps[:], wq_c[:, kc * 128:(kc + 1) * 128],
                                     qTt[kc][:], start=(kc == 0),
                                     stop=(kc == 7))
                nc.vector.tensor_scalar(QT[ec][:], ps[:],
                                        bq128[:, ec:ec + 1], None, ALU.add)

            for ec in range(8):
                wk_c = wcol.tile([128, 1024], BF16, tag="wcol")
                nc.sync.dma_start(
                    wk_c[:],
                    d["wk"][:, ec * 128:(ec + 1) * 128]
                    .rearrange("(k p) c -> p k c", p=128))
                ps = p1ps.tile([128, 512], F32, tag="p1", name="kvps")
                for kc in range(8):
                    nc.tensor.matmul(ps[:], wk_c[:, kc * 128:(kc + 1) * 128],
                                     kTt[kc][:], start=(kc == 0),
                                     stop=(kc == 7))
                nc.vector.tensor_scalar(KT[ec][:], ps[:],
                                        bk128[:, ec:ec + 1], None, ALU.add)

            # V projection: V[s, e] tiles; lhsT = vT column-blocks.
            for st in range(4):
                vt_c = wcol.tile([128, 1024], BF16, tag="wcol")
                nc.sync.dma_start(
                    vt_c[:],
                    d["vT"][:, st * 128:(st + 1) * 128]
                    .rearrange("(k p) c -> p k c", p=128))
                for et in range(2):
                    ps = p1ps.tile([128, 512], F32, tag="p1", name="kvps")
                    for kc in range(8):
                        nc.tensor.matmul(
                            ps[:], vt_c[:, kc * 128:(kc + 1) * 128],
                            wv_r[kc][:, et * 512:(et + 1) * 512],
                            start=(kc == 0), stop=(kc == 7))
                    nc.vector.tensor_tensor(
                        Vsb[st][et][:], ps[:],
                        bvsb[:, et * 512:(et + 1) * 512], ALU.add)
            if debug:
                nc.sync.dma_start(dbg["dQT"][:], QT[0][:].bitcast(F32))
                nc.sync.dma_start(dbg["dKT"][:], KT[0][:].bitcast(F32))
                nc.sync.dma_start(dbg["dV"][:], Vsb[0][0][:])
                nc.sync.dma_start(dbg["dhjT"][:], hjT[:])
                nc.sync.dma_start(dbg["dhiT"][:], hiT[:])

        # ---------------- phase 3: scores + softmax + AV ----------------
        # attn = exp(scores) * ebias; softmax denominators via one-hot
        # matmuls; AV on the biased weights.
        with tc.tile_pool(name="attnT", bufs=3) as attp, \
             tc.tile_pool(name="attnP", bufs=3) as atpp, \
             tc.tile_pool(name="scps", bufs=3, space="PSUM") as scps, \
             tc.tile_pool(name="smps", bufs=2, space="PSUM") as smps, \
             tc.tile_pool(name="avps", bufs=2, space="PSUM") as avps, \
             tc.tile_pool(name="rbps", bufs=1, space="PSUM") as rbps:

            # two sums groups (heads 0-7 / 8-15) so half the softmax
            # normalization + avN scaling runs mid-phase instead of in
            # the serialized tail
            sums_g = [smps.tile([16, NQ], F32, tag="sums", name=f"sums{g}")
                      for g in range(2)]
            av_ps = None

            def normalize_group(g):
                # heads of group g land in sums rows 0-7 (hsel remaps
                # h -> h%8); 1/sums there, then scale avU -> avN
                nc.vector.tensor_copy(sums_sb[0:8, :], sums_g[g][0:8, :])
                with nc.allow_low_precision(reason="f32r bits are fp32"):
                    nc.vector.reciprocal(recip_sb[0:8, :],
                                         sums_sb[0:8, :])
                for hp in range(4 * g, 4 * g + 4):
                    bc_ps = rbps.tile([128, NQ], F32, tag="bc")
                    nc.tensor.matmul(bc_ps[:],
                                     rsel[:, hp * 128:(hp + 1) * 128],
                                     recip_sb[0:8, :],
                                     start=True, stop=True)
                    nc.vector.tensor_tensor(avN[hp][:], avU[hp][:],
                                            bc_ps[:], ALU.mult)

            def consume(h, atp_big):
                nonlocal av_ps
                hp, hw = h // 2, (h % 2) * 64
                g = h // 8
                if h % 2 == 0:
                    av_ps = avps.tile([128, NQ], F32, tag="av")
                for jc in range(4):
                    nc.tensor.matmul(
                        sums_g[g][:], hsel[:, h * 16:(h + 1) * 16],
                        atp_big[:, jc * NQ:(jc + 1) * NQ],
                        start=(h % 8 == 0 and jc == 0),
                        stop=(h % 8 == 7 and jc == 3), skip_group_check=True)
                for jc in range(4):
                    st, et = jc, h // 8
                    nc.tensor.matmul(
                        av_ps[hw:hw + 64, :],
                        Vsb[st][et][:, (h % 8) * 64:(h % 8) * 64 + 64],
                        atp_big[:, jc * NQ:(jc + 1) * NQ],
                        start=(jc == 0), stop=(jc == 3),
                        skip_group_check=True,
                        tile_position=(0, hw))
                if h % 2 == 1:
                    nc.vector.tensor_copy(avU[hp][:], av_ps[:])
                if h % 8 == 7:
                    normalize_group(g)

            for h in range(16):
                hp, hw = h // 2, (h % 2) * 64
                at_big = attp.tile([128, 4 * NQ], BF16, tag="at")
                atp_big = atpp.tile([128, 4 * NQ], BF16, tag="atp")
                for jp in range(2):
                    sc_ps = scps.tile([128, 2 * NQ], F32, tag="sc")
                    for k in range(2):
                        jc = 2 * jp + k
                        nc.tensor.matmul(
                            sc_ps[:, k * NQ:(k + 1) * NQ],
                            KT[hp][hw:hw + 64, jc * 128:(jc + 1) * 128],
                            QT[hp][hw:hw + 64, :],
                            start=True, stop=True)
                    nc.scalar.activation(
                        at_big[:, jp * 2 * NQ:(jp + 1) * 2 * NQ],
                        sc_ps[:], AF.Exp)
                    bview = biasT[:].rearrange(
                        "p (j x) -> p j x",
                        j=4)[:, 2 * jp:2 * jp + 2, h * 256:(h + 1) * 256]
                    nc.vector.tensor_tensor(
                        atp_big[:, jp * 2 * NQ:(jp + 1) * 2 * NQ],
                        at_big[:, jp * 2 * NQ:(jp + 1) * 2 * NQ],
                        bview, ALU.mult)
                consume(h, atp_big)

            if debug:
                nc.sync.dma_start(dbg["dsums"][:], sums_sb[:])
                nc.sync.dma_start(dbg["davU"][:], avU[0][:])
                nc.sync.dma_start(dbg["davN"][:], avN[0][:])

        # ---------------- phase 4: output projection ----------------
        # hp-outer so the first accumulation steps start as soon as
        # avN[0] lands, overlapping the normalization tail.
        with tc.tile_pool(name="osb", bufs=2) as osb, \
             tc.tile_pool(name="ops", bufs=2, space="PSUM") as ops:
            pss = [ops.tile([128, 1024], F32, tag="ops", name=f"ops{g}")
                   for g in range(2)]
            for hp in range(8):
                for ttile in range(2):
                    for et in range(2):
                        nc.tensor.matmul(
                            pss[ttile][:, et * 512:(et + 1) * 512],
                            avN[hp][:, ttile * 128:(ttile + 1) * 128],
                            wo_r[hp][:, et * 512:(et + 1) * 512],
                            start=(hp == 0), stop=(hp == 7))
            for ttile in range(2):
                o_sb = osb.tile([128, 1024], F32, tag="osb")
                for et in range(2):
                    # halves evict on different engines and store on
                    # different DMA queues so the final writes overlap
                    half = slice(et * 512, (et + 1) * 512)
                    if et == 0:
                        nc.scalar.copy(o_sb[:, half], pss[ttile][:, half])
                        nc.sync.dma_start(
                            out_d[ttile * 128:(ttile + 1) * 128, half],
                            o_sb[:, half])
                    else:
                        nc.vector.tensor_copy(o_sb[:, half],
                                              pss[ttile][:, half])
                        nc.gpsimd.dma_start(
                            out_d[ttile * 128:(ttile + 1) * 128, half],
                            o_sb[:, half])

    nc.compile()
    return nc


def _host_prep(inputs):
    """Build the 8 per-core input maps from the full inputs."""
    f32 = np.float32
    q = np.ascontiguousarray(inputs["query"], f32)
    k = np.ascontiguousarray(inputs["key"], f32)
    v = np.ascontiguousarray(inputs["value"], f32)
    se = np.ascontiguousarray(inputs["state_embeddings"], f32)
    scale = f32(D) ** f32(-0.5)
    wq = np.ascontiguousarray(inputs["Wq"] * scale).astype(_BF)
    wk = np.ascontiguousarray(inputs["Wk"]).astype(_BF)
    wv = np.ascontiguousarray(inputs["Wv"]).astype(_BF)
    wo = np.ascontiguousarray(inputs["Wo"]).astype(_BF)
    bq = np.asarray(inputs["bq"], f32) * scale
    bk = np.asarray(inputs["bk"], f32)
    bv = np.asarray(inputs["bv"], f32)
    w1 = np.asarray(inputs["W1"], f32)
    b1 = np.asarray(inputs["b1"], f32)
    w2 = np.asarray(inputs["W2"], f32)
    b2 = np.asarray(inputs["b2"], f32)
    w3 = np.asarray(inputs["W3"], f32)
    b3 = np.asarray(inputs["b3"], f32)
    temps = np.asarray(inputs["head_temps"], f32)

    w1a_dup = np.concatenate([w1[:SD], w1[:SD]], axis=1)          # [64,128]
    w1b_dup = np.concatenate([w1[SD:], w1[SD:]], axis=1)          # [64,128]
    w2bd = np.zeros((128, 128), f32)
    w2bd[:64, :64] = w2
    w2bd[64:, 64:] = w2
    # w3bd columns ordered (head x, parity m): col = 2*x + m, so the
    # transposed comp blocks land as (i, x, m) and phase-3 per-(h,jc)
    # bias views are contiguous.
    w3bd = np.zeros((128, 32), f32)
    w3bd[:64, 0::2] = w3         # m = 0 parity
    w3bd[64:, 1::2] = w3         # m = 1 parity
    # head h's softmax sum lands in row h%8 of its group's PSUM tile
    hsel = np.zeros((128, H * 16), f32)
    for h in range(H):
        hsel[:, h * 16 + (h % 8)] = 1.0
    rsel = np.zeros((8, 8 * 128), f32)
    for hp in range(8):
        rsel[(2 * hp) % 8, hp * 128:hp * 128 + 64] = 1.0
        rsel[(2 * hp + 1) % 8, hp * 128 + 64:hp * 128 + 128] = 1.0
    # c_ps partition order within each 32-block is (x, m): value temps[x]
    t128 = np.tile(np.repeat(temps, 2), 4).reshape(128, 1)
    b3t = np.tile(np.repeat(b3 * temps, 2), 4).reshape(128, 1)
    b1d = np.tile(b1, 2).reshape(128, 1)
    b2d = np.tile(b2, 2).reshape(128, 1)
    bq128 = bq.reshape(8, 128).T.copy()
    bk128 = bk.reshape(8, 128).T.copy()
    id128 = np.eye(128, dtype=f32).astype(_BF)
    bv2d = bv.reshape(1, E).astype(_BF)

    shared = dict(wq=wq, wk=wk, wv=wv, wo=wo, w1a=w1a_dup, w1b=w1b_dup,
                  w2bd=w2bd.astype(_BF), w3bd=w3bd.astype(_BF),
                  id128=id128, hsel=hsel.astype(_BF), bq128=bq128, bk128=bk128,
                  b1d=b1d, b2d=b2d, t128=t128, b3t=b3t, bv2d=bv2d,
                  rsel=rsel)
    maps = []
    for c in range(NCORES):
        b, half = c // 2, c % 2
        rows = slice(half * NQ, (half + 1) * NQ)
        m = dict(shared)
        m["qT"] = np.ascontiguousarray(q[b, rows].T).astype(_BF)
        m["kT"] = np.ascontiguousarray(k[b].T).astype(_BF)
        m["vT"] = np.ascontiguousarray(v[b].T).astype(_BF)
        m["seT"] = np.ascontiguousarray(se[b].T)
        m["seQ"] = np.ascontiguousarray(se[b, rows].T)
        maps.append(m)
    return maps


_cache = {}


def _get_program():
    if "nc" not in _cache:
        _cache["nc"] = _build_program()
    return _cache["nc"]


def kernel(**inputs):
    nc = _get_program()
    maps = _host_prep(inputs)
    res = run_bass_kernel_spmd(nc, maps, list(range(NCORES)))
    bo = np.asarray(inputs["bo"], np.float32)
    out = np.empty((B, N, E), np.float32)
    for c in range(NCORES):
        b, half = c // 2, c % 2
        out[b, half * NQ:(half + 1) * NQ] = res.results[c]["out"]
    return out + bo

